# revision 5
# baseline (speedup 1.0000x reference)
"""Causal single-head attention (B=8, N=2048, D=H=1024, fp32) on 8 TRN2 cores.

Data-parallel: one batch element per NeuronCore. All matmuls run in fp16
(PE upconverts to its internal fp22 pipeline, so fp16 keeps 10 mantissa bits
and streams at 1 cycle/column like fp32r, but halves DMA/SBUF), and the whole
working set (x^T, Q^T, K^T, V) stays resident in SBUF — no DRAM spills.

Layouts (per core, partition dim first):
  xt [128, 8(db), 2048(n)]   x^T, d-blocked
  wq/wk [8(hb), 128, 8(db), 128]  weight tiles, h-block-major for DMA gating
  wv [128, 8(db), 1024(h)]
  qt/kt [128, 8(hb), 2048(n)]  Q^T/K^T resident
  vp [128, 16(nb), 1024(h)]    V resident
Scores are computed transposed (S^T = K^T_slice.T @ Q^T tile, [key, query]);
softmax normalization is folded into the output eviction as a per-partition
scale; causal masking: partial-width diagonal score groups + a triangular
mask multiply (DVE) on the exact-diagonal 128x128 sub-block.
"""

import os
import sys
from contextlib import ExitStack

import numpy as np

try:
    import concourse.bacc as bacc
except ImportError:  # pragma: no cover
    sys.path.insert(0, "/opt/trn_rl_repo")
    import concourse.bacc as bacc

import concourse.mybir as mybir
from concourse.tile import TileContext
from concourse.bass_utils import run_bass_kernel_spmd

# bass_utils imports antenv.axon_hooks when BASS_TRACE is set; provide a stub
# so tracing degrades gracefully instead of crashing if the module is absent.
try:
    import antenv.axon_hooks  # noqa: F401
except ImportError:  # pragma: no cover
    import types

    _m = types.ModuleType("antenv.axon_hooks")
    _m._hook = None
    _m.set_axon_ntff_profile_hook = lambda h: setattr(_m, "_hook", h)
    _m.get_axon_ntff_profile_hook = lambda: _m._hook
    sys.modules["antenv.axon_hooks"] = _m

B, N, D, H = 8, 2048, 1024, 1024
P = 128
DT = D // P          # 8 contraction tiles for the projections
HT = H // P          # 8 h-tiles
NT = N // P          # 16 sequence tiles of 128
IT = N // 512        # 4 query tiles of 512
SCALE = 1.0 / np.sqrt(float(H))

F32 = mybir.dt.float32
FP16 = mybir.dt.float16

LAST_RESULT = None  # BassKernelResults of the most recent kernel() call
_CACHE = {}


def build_program():
    nc = bacc.Bacc("TRN2", target_bir_lowering=False, debug=False)

    xt_d = nc.dram_tensor("xt", [P, DT, N], FP16, kind="ExternalInput")
    wq_d = nc.dram_tensor("wq", [P, HT, DT, P], FP16, kind="ExternalInput")
    wk_d = nc.dram_tensor("wk", [P, HT, DT, P], FP16, kind="ExternalInput")
    wv_d = nc.dram_tensor("wv", [P, DT, H], FP16, kind="ExternalInput")
    bq_d = nc.dram_tensor("bqT", [P, HT], F32, kind="ExternalInput")
    bk_d = nc.dram_tensor("bkT", [P, HT], F32, kind="ExternalInput")
    bv_d = nc.dram_tensor("bvB", [P, H], F32, kind="ExternalInput")
    tri_d = nc.dram_tensor("tri", [P, P], FP16, kind="ExternalInput")
    # fp16 output staging halves the output DMA traffic; the host upcasts to
    # fp32 (adds ~1e-4 rel err, far under budget).
    out = nc.dram_tensor("out", [N, H], FP16, kind="ExternalOutput")

    Exp = mybir.ActivationFunctionType.Exp
    Identity = mybir.ActivationFunctionType.Identity
    Copy = mybir.ActivationFunctionType.Copy

    with TileContext(nc) as tc:
        with ExitStack() as top:
            const = top.enter_context(tc.tile_pool(name="const", bufs=1))
            qt_pool = top.enter_context(tc.tile_pool(name="qt", bufs=1))
            kt_pool = top.enter_context(tc.tile_pool(name="kt", bufs=1))
            vp_pool = top.enter_context(tc.tile_pool(name="vp", bufs=1))

            ones = const.tile([P, 1], FP16, tag="ones")
            nc.vector.memset(ones[:], 1.0)
            tri = const.tile([P, P], FP16, tag="tri")
            bq_sb = const.tile([P, HT], F32, tag="bq")
            bk_sb = const.tile([P, HT], F32, tag="bk")
            bv_sb = const.tile([P, H], F32, tag="bv")

            qt = qt_pool.tile([P, HT, N], FP16, tag="qt")
            kt = kt_pool.tile([P, HT, N], FP16, tag="kt")
            vp = vp_pool.tile([P, NT, H], FP16, tag="vp")

            # ---------------- Phase 1: projections (Q, K, then V) ----------
            with ExitStack() as p1:
                xt_pool = p1.enter_context(tc.tile_pool(name="xt", bufs=1))
                w_pool = p1.enter_context(tc.tile_pool(name="w", bufs=1))
                ps1 = p1.enter_context(tc.tile_pool(name="ps1", bufs=6, space="PSUM"))

                xt = xt_pool.tile([P, DT, N], FP16, tag="xt")
                wq = w_pool.tile([P, HT, DT, P], FP16, tag="wq")
                wk = w_pool.tile([P, HT, DT, P], FP16, tag="wk")
                wv = w_pool.tile([P, DT, H], FP16, tag="wv")

                # DMA priority order: biases/tri first (evictions read them —
                # if they land late every psum eviction blocks and the PE
                # stalls once the banks fill), then the first Q group's
                # gating chunks (wq hb0 + xt nch0, interleaved per-db so the
                # first matmuls' operands dispatch earliest), then the rest.
                nc.sync.dma_start(bq_sb[:], bq_d.ap()[:, :])
                nc.sync.dma_start(bk_sb[:], bk_d.ap()[:, :])
                nc.sync.dma_start(tri[:], tri_d.ap()[:, :])
                for db in range(DT):
                    nc.sync.dma_start(wq[:, 0, db, :], wq_d.ap()[:, 0, db, :])
                    nc.sync.dma_start(xt[:, db, 0:512], xt_d.ap()[:, db, 0:512])

                def load_w_hb(w_sb, w_dram, hb):
                    # whole h-block in one DMA: [128, 8, 128] fp16 is
                    # contiguous per partition (2KB lines vs 256B per-db)
                    nc.sync.dma_start(
                        w_sb[:, hb, :, :], w_dram.ap()[:, hb, :, :]
                    )

                for hb in range(1, HT):
                    load_w_hb(wq, wq_d, hb)
                for nch in range(1, 4):
                    for db in range(DT):
                        nc.sync.dma_start(
                            xt[:, db, 512 * nch : 512 * (nch + 1)],
                            xt_d.ap()[:, db, 512 * nch : 512 * (nch + 1)],
                        )
                for hch in range(2):
                    nc.sync.dma_start(
                        bv_sb[:, 512 * hch : 512 * (hch + 1)],
                        bv_d.ap()[:, 512 * hch : 512 * (hch + 1)],
                    )
                for hb in range(HT):
                    load_w_hb(wk, wk_d, hb)
                for db in range(DT):
                    nc.sync.dma_start(wv[:, db, :], wv_d.ap()[:, db, :])

                # Q^T and K^T interleaved by n-chunk: psum group per
                # (nch, proj, hb); halves the xt arrival-deadline pressure
                # and overlaps the wk load with Q compute.  Evictions
                # alternate scalar/vector engines, writing into qt/kt.
                def proj_group(wtile, dst, bias, nch, hb, eng):
                    n0 = 512 * nch
                    ps = ps1.tile([P, 512], F32, tag="ps")
                    for db in range(DT):
                        nc.tensor.matmul(
                            ps[:],
                            wtile[:, hb, db, :],
                            xt[:, db, n0 : n0 + 512],
                            start=(db == 0),
                            stop=(db == DT - 1),
                        )
                    d = dst[:, hb, n0 : n0 + 512]
                    if eng == 0:
                        nc.scalar.activation(
                            d, ps[:], Identity, bias=bias[:, hb : hb + 1]
                        )
                    else:
                        nc.vector.tensor_scalar_add(d, ps[:], bias[:, hb : hb + 1])

                for nch in range(4):
                    for hb in range(HT):
                        proj_group(wq, qt, bq_sb, nch, hb, (nch + hb) % 2)
                for nch in range(4):
                    for hb in range(HT):
                        proj_group(wk, kt, bk_sb, nch, hb, (nch + hb) % 2)

                # V = x @ Wv (+ bv): psum group per (nb, hch); stationary is
                # the x^T block, Wv streams.
                for nb in range(NT):
                    for hch in range(2):
                        h0 = 512 * hch
                        ps = ps1.tile([P, 512], F32, tag="ps")
                        for db in range(DT):
                            nc.tensor.matmul(
                                ps[:],
                                xt[:, db, nb * P : (nb + 1) * P],
                                wv[:, db, h0 : h0 + 512],
                                start=(db == 0),
                                stop=(db == DT - 1),
                            )
                        nc.vector.tensor_add(
                            vp[:, nb, h0 : h0 + 512],
                            ps[:],
                            bv_sb[:, h0 : h0 + 512],
                        )

            # ---------------- Phase 2: attention ----------------
            with ExitStack() as p2:
                pt_pool = p2.enter_context(tc.tile_pool(name="pt", bufs=2))
                ot_pool = p2.enter_context(tc.tile_pool(name="op", bufs=4))
                sm_pool = p2.enter_context(tc.tile_pool(name="sm", bufs=4))
                ps_s = p2.enter_context(tc.tile_pool(name="pss", bufs=3, space="PSUM"))
                ps_av = p2.enter_context(tc.tile_pool(name="psav", bufs=4, space="PSUM"))
                ps_rs = p2.enter_context(tc.tile_pool(name="psrs", bufs=1, space="PSUM"))

                for t in range(IT):
                    i0 = 512 * t
                    jmax = 4 * t + 3

                    # scores^T [key j, query i] + exp (+ diag tri-mask)
                    pt = []
                    for j in range(jmax + 1):
                        c = max(0, j * P - i0)
                        w_ = 512 - c
                        ps = ps_s.tile([P, 512], F32, tag="ps")
                        for hb in range(HT):
                            nc.tensor.matmul(
                                ps[:, 0:w_],
                                kt[:, hb, j * P : (j + 1) * P],
                                qt[:, hb, i0 + c : i0 + 512],
                                start=(hb == 0),
                                stop=(hb == HT - 1),
                            )
                        p = pt_pool.tile([P, 512], FP16, tag=f"pt{j}", name=f"pt{j}")
                        nc.scalar.activation(
                            p[:, c:512], ps[:, 0:w_], Exp, scale=float(SCALE)
                        )
                        if c > 0 or j * P == i0:
                            # exact-diagonal 128x128 sub-block: keep key<=query
                            nc.vector.tensor_mul(
                                p[:, c : c + P], p[:, c : c + P], tri[:]
                            )
                        pt.append(p)

                    # attn @ V + row-sums; normalization folded into eviction
                    for s in range(4):
                        g = 4 * t + s
                        pav = [
                            ps_av.tile([P, 512], F32, tag="pav", name="pav")
                            for _ in range(2)
                        ]
                        prs = ps_rs.tile([P, 1], F32, tag="prs")
                        for j in range(g + 1):
                            lhsT = pt[j][:, s * P : (s + 1) * P]
                            for hch in range(2):
                                nc.tensor.matmul(
                                    pav[hch][:],
                                    lhsT,
                                    vp[:, j, 512 * hch : 512 * (hch + 1)],
                                    start=(j == 0),
                                    stop=(j == g),
                                )
                            nc.tensor.matmul(
                                prs[:],
                                lhsT,
                                ones[:],
                                start=(j == 0),
                                stop=(j == g),
                            )
                        recip = sm_pool.tile([P, 1], F32, tag="recip")
                        nc.vector.reciprocal(recip[:], prs[:])
                        ot = ot_pool.tile([P, H], FP16, tag="ot")
                        nc.scalar.activation(
                            ot[:, 0:512], pav[0][:], Copy, scale=recip[:]
                        )
                        nc.vector.tensor_scalar_mul(
                            ot[:, 512:1024], pav[1][:], recip[:]
                        )
                        r0 = i0 + s * P
                        # the very last block splits 4-way so its final
                        # transfer (the span tail) is shorter
                        nsplit = 4 if (t == IT - 1 and s == 3) else 1
                        w_o = H // nsplit
                        for oc in range(nsplit):
                            nc.sync.dma_start(
                                out.ap()[r0 : r0 + P, w_o * oc : w_o * (oc + 1)],
                                ot[:, w_o * oc : w_o * (oc + 1)],
                            )

    nc.compile()
    return nc


def _get_program():
    if "v2" not in _CACHE:
        _CACHE["v2"] = build_program()
    return _CACHE["v2"]


def kernel(x, Wq, bq, Wk, bk, Wv, bv):
    global LAST_RESULT
    x = np.asarray(x, dtype=np.float32)
    f16 = np.float16

    def prep_w_hb(W):
        # [D, H] -> [p, hb, db, hc] fp16
        return np.ascontiguousarray(
            np.asarray(W, np.float32)
            .reshape(DT, P, HT, P)
            .transpose(1, 2, 0, 3)
            .astype(f16)
        )

    def prep_wv(W):
        # [D, H] -> [p, db, h] fp16
        return np.ascontiguousarray(
            np.asarray(W, np.float32).reshape(DT, P, H).transpose(1, 0, 2).astype(f16)
        )

    xt_b = [
        np.ascontiguousarray(
            x[b].T.reshape(DT, P, N).transpose(1, 0, 2).astype(f16)
        )
        for b in range(B)
    ]
    wq_c = prep_w_hb(Wq)
    wk_c = prep_w_hb(Wk)
    wv_c = prep_wv(Wv)
    bqT = np.ascontiguousarray(np.asarray(bq, np.float32).reshape(HT, P).T)
    bkT = np.ascontiguousarray(np.asarray(bk, np.float32).reshape(HT, P).T)
    bvB = np.ascontiguousarray(
        np.broadcast_to(np.asarray(bv, np.float32), (P, H))
    )
    tri = np.ascontiguousarray(np.triu(np.ones((P, P), dtype=f16)))

    nc = _get_program()
    in_maps = [
        {
            "xt": xt_b[b],
            "wq": wq_c,
            "wk": wk_c,
            "wv": wv_c,
            "bqT": bqT,
            "bkT": bkT,
            "bvB": bvB,
            "tri": tri,
        }
        for b in range(B)
    ]

    res = run_bass_kernel_spmd(nc, in_maps, core_ids=list(range(B)))
    LAST_RESULT = res
    return np.stack([res.results[b]["out"] for b in range(B)], axis=0).astype(
        np.float32
    )
